# revision 19
# baseline (speedup 1.0000x reference)
"""Trainium2 Bass kernel for nn_MultiHeadHighLevelAllocator.

Math (reference):
    ue = MLP3(uav_feat)                            # (B,U,E)
    te = MLP3(task_feat)                           # (B,T,E)
    q  = ue[:,None,:,:] + head_q[None,:,None,:]    # (B,H,U,E)
    logits[b,h,u,t] = relu(q[b,h,u]@Wq + te[b,t]@Wk + fb1) @ fw2 + fb2

Key decomposition: by linearity of the projections,
    pre[b,h,u,t,:] = base[b,u,t,:] + hqP[h,:]
where base[b,u,t,:] = ue[b,u]@Wq + te[b,t]@Wk  (outer sum, H-independent)
and   hqP[h,:] = head_q[h]@Wq + fb1.

Per-core (data parallel over B, 2 batches/core):
  1. Encoders on TensorE in transposed layout (feat x rows), ScalarE ReLU+bias.
  2. base tiles (128d x 512) produced by two accumulating matmuls into PSUM
     (stride-0 broadcast APs replicate ue columns over t / te block over u).
  3. Per head: ReLU(base + hqP[h]) with per-partition bias -> fp16 tiles
     (VectorE tensor_scalar add+max for 2 heads, ScalarE activation for 2).
  4. Reduction against fw2 via masked-stationary matmuls: a (128x32) fp16
     stationary holding fw2-chunk in column j writes the dot product row to
     PSUM partition 32g+j of strip-g's own bank, accumulating zeros elsewhere;
     16 u-blocks x 2 heads x 2 chunks accumulate per strip bank group.
  5. One (128x512) fp32 result tile (+fb2) DMAed out per core.

All per-core inputs are packed host-side into a single (128, 3727) fp32
tensor loaded by ONE DMA (PE instructions only support a single sync wait,
so first-use deps must collapse to one semaphore).
"""
import os
import sys

for _p in ("/opt/trn_rl_repo", "/root/.axon_site/_ro/trn_rl_repo"):
    if os.path.isdir(_p) and _p not in sys.path:
        sys.path.insert(0, _p)

import numpy as np
import concourse.bass as bass
import concourse.mybir as mybir
from concourse import tile

B, U, T = 16, 64, 128
UAV_DIM, TASK_DIM = 32, 32
E, H, HID = 128, 4, 256
ENC_H = 128
NCORES = 8
BL = B // NCORES          # batches per core
NBLK = U // 4             # 16 u-blocks of 4 us -> N=512 columns each
f32, f16 = mybir.dt.float32, mybir.dt.float16
bf16 = mybir.dt.bfloat16
AF = mybir.ActivationFunctionType
ALU = mybir.AluOpType

# packed constant-tensor column layout (fp32 columns)
_C_UAVT = 0          # (32, 128)
_C_TASKT = 128       # (32, 256)
_C_UW0 = 384         # (32, 128)
_C_TW0 = 512         # (32, 128)
_C_UW1 = 640         # (128, 128)
_C_UW2 = 768
_C_TW1 = 896
_C_TW2 = 1024
_C_ENCB = 1152       # (128, 7): ub0 ub1 ub2 tb0 tb1 tb2 fb2
_C_HQPB = 1159       # (128, 8): col c*4+h
_C_WQK = 1167        # (128, 512): Wq c0 | Wq c1 | Wk c0 | Wk c1
# wz: two 63-col fp16 segments; fw2 chunk c at col c*63+31. The (128x32)
# masked stationary with fw2 at column j is the window [c*63+31-j, +32).
_C_WZ = 1679
_C_TOTAL = 1805

_BUILD_PAT = ["G", "D", "A", "G", "D", "A", "G", "A",
              "G", "D", "A", "G", "G", "D", "A", "G"]

_cache: dict = {}


def _split_multi_waits(nc):
    """Walrus in this toolchain rejects >1 sync wait per engine instruction
    ("Too many sync wait commands"). Hoist extra waits onto preceding
    same-engine NoOps — identical semantics on the in-order engine queues."""
    n_split = 0
    for func in nc.m.functions:
        for bb in func.blocks:
            new = []
            for ins in bb.instructions:
                si = ins.sync_info
                waits = list(si.on_wait) if (si and si.on_wait) else []
                if len(waits) > 1:
                    for k, w in enumerate(waits[:-1]):
                        nop = mybir.InstNoOp(name=f"{ins.name}_hw{k}", ins=[], outs=[])
                        nop.engine = ins.engine
                        nop.sync_info = mybir.SyncInfo(on_wait=[w], on_update=[])
                        new.append(nop)
                        n_split += 1
                    si.on_wait = [waits[-1]]
                new.append(ins)
            bb.instructions = new
    return n_split


def _build_nc():
    nc = bass.Bass()
    packed = nc.dram_tensor("packed", [128, _C_TOTAL], f32, kind="ExternalInput")
    out = nc.dram_tensor("out", [128, 512], f32, kind="ExternalOutput")

    with tile.TileContext(nc) as tc:
        with (
            tc.tile_pool(name="const", bufs=1) as constp,
            tc.tile_pool(name="persist", bufs=1) as persistp,
            tc.tile_pool(name="encw", bufs=2) as encwp,
        ):
            A = constp.tile([128, _C_TOTAL], f32, tag="all")
            nc.gpsimd.dma_start(A[:, :_C_WQK], packed[:, :_C_WQK])
            nc.gpsimd.dma_start(A[:, _C_WQK:], packed[:, _C_WQK:])
            # fp32 -> fp16 conversion on ScalarE: also serves as ScalarE's
            # first touch of the DMA'd tile, so later ACT instructions never
            # pair a DMA-sem wait with an engine-sem wait (ISA wait-slot
            # limits; PE matmuls only support a single wait).
            # wz holds fw2 chunk c in column c*63+31; the (128x32) masked
            # stationary with fw2 at column j is the window [c*63+31-j, +32).
            act_touch = constp.tile([128, 1], f32, tag="acttouch")
            nc.scalar.copy(act_touch[:], A[:, 0:1])
            sb_wz = constp.tile([128, 126], f16, tag="wz")
            nc.scalar.copy(sb_wz[:], A[:, _C_WZ:_C_WZ + 126])
            # VectorE first touch of the DMA'd tile (same wait-slot reason).
            dve_touch = constp.tile([128, 1], f32, tag="dvetouch")
            nc.vector.tensor_copy(dve_touch[:], A[:, 0:1])
            # fp16 projection weights: fp32 moving operands stream at half
            # rate through the PE array, so the base matmuls run fp16.
            sb_wqk16 = constp.tile([128, 512], f16, tag="wqk16")
            nc.scalar.copy(sb_wqk16[:], A[:, _C_WQK:_C_WQK + 512])

            enc_w = {
                "uw0": A[0:32, _C_UW0:_C_UW0 + 128],
                "tw0": A[0:32, _C_TW0:_C_TW0 + 128],
                "uw1": A[:, _C_UW1:_C_UW1 + 128],
                "uw2": A[:, _C_UW2:_C_UW2 + 128],
                "tw1": A[:, _C_TW1:_C_TW1 + 128],
                "tw2": A[:, _C_TW2:_C_TW2 + 128],
            }

            def encb_col(i):
                return A[:, _C_ENCB + i:_C_ENCB + i + 1]

            # ---- pools for the whole kernel (8 PSUM banks exactly:
            #      encoder 1 + base 3 + logits 4) ----
            with (
                tc.tile_pool(name="bsbp", bufs=2) as bsbp,
                tc.tile_pool(name="relup", bufs=4) as relup,
                tc.tile_pool(name="outp", bufs=1) as outp,
                tc.tile_pool(name="bpp", bufs=2, space="PSUM") as psB,
                tc.tile_pool(name="lpp", bufs=1, space="PSUM") as psL,
            ):
                # ---- encoders: chain in (feat x rows) layout ----
                def mlp3(xT, rows, wnames, bcols, tag):
                    cur = xT
                    for li in range(3):
                        ps = psB.tile([128, 512], f32, tag="bp", name=f"ps{tag}{li}")
                        ps = ps[:, :rows]
                        nc.tensor.matmul(ps[:], enc_w[wnames[li]], cur,
                                         start=True, stop=True)
                        if li < 2:
                            nxt = encwp.tile([128, rows], f32, tag=f"{tag}h",
                                             name=f"{tag}h{li}")
                            nc.scalar.activation(nxt[:], ps[:], AF.Relu,
                                                 bias=encb_col(bcols[li]), scale=1.0)
                        else:
                            nxt = persistp.tile([128, rows], f16, tag=f"{tag}T",
                                                name=f"{tag}T")
                            nc.scalar.activation(nxt[:], ps[:], AF.Identity,
                                                 bias=encb_col(bcols[li]), scale=1.0)
                        cur = nxt[:]
                    return cur

                ueT = mlp3(A[0:32, _C_UAVT:_C_UAVT + BL * U], BL * U,
                           ("uw0", "uw1", "uw2"), (0, 1, 2), "ue")
                teT = mlp3(A[0:32, _C_TASKT:_C_TASKT + BL * T], BL * T,
                           ("tw0", "tw1", "tw2"), (3, 4, 5), "te")

                lp = [psL.tile([128, 512], f32, tag=f"lp{g}", name=f"lp{g}")
                      for g in range(4)]
                bi = 0
                NS = 8          # n-blocks per relu slab (FD = NS*512)
                for b in range(BL):
                    for c in range(2):
                        # small projections for this (batch, chunk):
                        # khP[d,t] = (te @ Wk_c)^T,  qP[d,u] = (ue @ Wq_c)^T
                        pk = psB.tile([128, 512], f32, tag="bp", name="pk")
                        nc.tensor.matmul(pk[:, :T],
                                         sb_wqk16[:, 256 + c * 128:256 + (c + 1) * 128],
                                         teT[:, b * T:(b + 1) * T],
                                         start=True, stop=True)
                        pq = psB.tile([128, 512], f32, tag="bp", name="pq")
                        nc.tensor.matmul(pq[:, :U],
                                         sb_wqk16[:, c * 128:(c + 1) * 128],
                                         ueT[:, b * U:(b + 1) * U],
                                         start=True, stop=True)
                        khP = bsbp.tile([128, T], f16, tag="khP", name="khP")
                        nc.scalar.copy(khP[:], pk[:, :T])
                        qP = bsbp.tile([128, U], f32, tag="qP", name="qP")
                        nc.scalar.copy(qP[:], pq[:, :U])
                        for n0 in range(0, NBLK, NS):
                            # base slab: base[d,(u,t)] = khP[d,t] + qP[d,u],
                            # built FD=128 at a time (u-specific bias), split
                            # between ScalarE and VectorE.
                            bsb = bsbp.tile([128, NS * 512], f16, tag="bsb",
                                            name="bsb")
                            # slab build: two 2048-col halves, each either
                            # one broadcast tensor_tensor (VectorE / GpSimd)
                            # or 16 per-u ScalarE bias-adds. 3-way split keeps
                            # all three producer engines busy.
                            for half in range(2):
                                dn0 = half * (NS // 2)
                                u0 = 4 * (n0 + dn0)
                                dst = bsb[:, dn0 * 512:(dn0 + NS // 2) * 512]
                                eng = _BUILD_PAT[bi % len(_BUILD_PAT)]
                                bi += 1
                                if eng != "A":
                                    nu = 2 * NS
                                    in0 = (khP[:].unsqueeze(1)
                                           .broadcast_to([128, nu, T]))
                                    in1 = (qP[:, u0:u0 + nu].unsqueeze(2)
                                           .broadcast_to([128, nu, T]))
                                    e = nc.vector if eng == "D" else nc.gpsimd
                                    e.tensor_tensor(dst, in0, in1, ALU.add)
                                else:
                                    for k in range(2 * NS):
                                        u = u0 + k
                                        nc.scalar.activation(
                                            bsb[:, (dn0 * 4 + k) * 128:
                                                (dn0 * 4 + k + 1) * 128],
                                            khP[:], AF.Identity,
                                            bias=qP[:, u:u + 1], scale=1.0)
                            for hp in range(2):
                                # heads hp and hp+2 land in different PE
                                # column groups (strips 2b, 2b+1): interleave
                                # their matmuls so the streams run
                                # concurrently in the array.
                                rts = {}
                                for h in (hp, hp + 2):
                                    rt = relup.tile([128, NS * 512], f16,
                                                    tag="rt", name="rt")
                                    bias_ap = A[:, _C_HQPB + c * 4 + h:
                                                _C_HQPB + c * 4 + h + 1]
                                    if h == 3:
                                        nc.scalar.activation(
                                            rt[:], bsb[:], AF.Relu,
                                            bias=bias_ap, scale=1.0)
                                    else:
                                        nc.vector.tensor_scalar(
                                            rt[:], bsb[:], bias_ap, 0.0,
                                            ALU.add, ALU.max)
                                    rts[h] = rt
                                for dn in range(NS):
                                    n = n0 + dn
                                    for h in (hp, hp + 2):
                                        p_ = (b * H + h) * NBLK + n
                                        g, j = p_ // 32, p_ % 32
                                        first = (c == 0 and n == 0
                                                 and h % 2 == 0)
                                        last = (c == 1 and n == NBLK - 1
                                                and h % 2 == 1)
                                        nc.tensor.matmul(
                                            lp[g][32 * g:32 * g + 32, :],
                                            sb_wz[:, c * 63 + 31 - j:
                                                  c * 63 + 63 - j],
                                            rts[h][:, dn * 512:(dn + 1) * 512],
                                            start=first, stop=last,
                                            tile_position=(0, 32 * g))

                sb_out = outp.tile([128, 512], f32, tag="sbout", name="sbout")
                for g in range(4):
                    nc.vector.tensor_scalar(
                        sb_out[32 * g:32 * g + 32, :],
                        lp[g][32 * g:32 * g + 32, :],
                        A[32 * g:32 * g + 32, _C_ENCB + 6:_C_ENCB + 7],
                        None, ALU.add)
                nc.sync.dma_start(out[:], sb_out[:])
    return nc


def _prep_inputs(uav_feat, task_feat, uw0, ub0, uw1, ub1, uw2, ub2,
                 tw0, tb0, tw1, tb1, tw2, tb2, head_q, fw1, fb1, fw2, fb2):
    f = np.float32
    uav = np.asarray(uav_feat, f)
    task = np.asarray(task_feat, f)
    fw1 = np.asarray(fw1, f)
    fw2 = np.asarray(fw2, f)
    Wq, Wk = fw1[:E], fw1[E:]

    base = np.zeros((128, _C_TOTAL), f)
    base[0:32, _C_UW0:_C_UW0 + 128] = np.asarray(uw0, f)
    base[0:32, _C_TW0:_C_TW0 + 128] = np.asarray(tw0, f)
    base[:, _C_UW1:_C_UW1 + 128] = np.asarray(uw1, f)
    base[:, _C_UW2:_C_UW2 + 128] = np.asarray(uw2, f)
    base[:, _C_TW1:_C_TW1 + 128] = np.asarray(tw1, f)
    base[:, _C_TW2:_C_TW2 + 128] = np.asarray(tw2, f)
    for i, v in enumerate((ub0, ub1, ub2, tb0, tb1, tb2)):
        base[:, _C_ENCB + i] = np.asarray(v, f)
    base[:, _C_ENCB + 6] = np.asarray(fb2, f)[0]
    hq = np.asarray(head_q, f) @ Wq + np.asarray(fb1, f)  # (H, HID)
    for c in range(2):
        for h in range(H):
            base[:, _C_HQPB + c * 4 + h] = hq[h, c * 128:(c + 1) * 128]
    base[:, _C_WQK:_C_WQK + 256] = Wq
    base[:, _C_WQK + 256:_C_WQK + 512] = Wk
    for c in range(2):
        base[:, _C_WZ + c * 63 + 31] = fw2[c * 128:(c + 1) * 128, 0]

    in_maps = []
    for k in range(NCORES):
        b0 = k * BL
        pk = base.copy()
        pk[0:32, _C_UAVT:_C_UAVT + BL * U] = \
            uav[b0:b0 + BL].reshape(BL * U, UAV_DIM).T
        pk[0:32, _C_TASKT:_C_TASKT + BL * T] = \
            task[b0:b0 + BL].reshape(BL * T, TASK_DIM).T
        in_maps.append({"packed": pk})
    return in_maps


def _gather(results):
    outs = []
    for k in range(NCORES):
        r = np.asarray(results[k]["out"], np.float32)  # (128, 512)
        outs.append(r.reshape(BL, H, NBLK, 4, T).reshape(BL, H, U, T))
    return np.concatenate(outs, axis=0)


def kernel(**inputs) -> np.ndarray:
    if "nc" not in _cache:
        _cache["nc"] = _build_nc()
    nc = _cache["nc"]
    in_maps = _prep_inputs(**inputs)
    if os.environ.get("BASS_KERNEL_SIM"):
        # CoreSim can't digest the hand-inserted wait-splitting NoOps; it
        # enforces the multi-wait semantics natively, so run unsplit.
        from concourse.bass_interp import CoreSim
        results = []
        for k in range(NCORES):
            sim = CoreSim(nc)
            for name, arr in in_maps[k].items():
                sim.tensor(name)[:] = arr
            sim.simulate()
            results.append({"out": np.array(sim.tensor("out"))})
    else:
        from concourse.bass_utils import run_bass_kernel_spmd
        if not _cache.get("split"):
            _split_multi_waits(nc)
            _cache["split"] = True
        results = run_bass_kernel_spmd(nc, in_maps, list(range(NCORES))).results
    return _gather(results)


# revision 20
# speedup vs baseline: 1.0741x; 1.0741x over previous
"""Trainium2 Bass kernel for nn_MultiHeadHighLevelAllocator.

Math (reference):
    ue = MLP3(uav_feat)                            # (B,U,E)
    te = MLP3(task_feat)                           # (B,T,E)
    q  = ue[:,None,:,:] + head_q[None,:,None,:]    # (B,H,U,E)
    logits[b,h,u,t] = relu(q[b,h,u]@Wq + te[b,t]@Wk + fb1) @ fw2 + fb2

Key decomposition: by linearity of the projections,
    pre[b,h,u,t,:] = base[b,u,t,:] + hqP[h,:]
where base[b,u,t,:] = ue[b,u]@Wq + te[b,t]@Wk  (outer sum, H-independent)
and   hqP[h,:] = head_q[h]@Wq + fb1.

Per-core (data parallel over B, 2 batches/core):
  1. Encoders on TensorE in transposed layout (feat x rows), ScalarE ReLU+bias.
  2. base tiles (128d x 512) produced by two accumulating matmuls into PSUM
     (stride-0 broadcast APs replicate ue columns over t / te block over u).
  3. Per head: ReLU(base + hqP[h]) with per-partition bias -> fp16 tiles
     (VectorE tensor_scalar add+max for 2 heads, ScalarE activation for 2).
  4. Reduction against fw2 via masked-stationary matmuls: a (128x32) fp16
     stationary holding fw2-chunk in column j writes the dot product row to
     PSUM partition 32g+j of strip-g's own bank, accumulating zeros elsewhere;
     16 u-blocks x 2 heads x 2 chunks accumulate per strip bank group.
  5. One (128x512) fp32 result tile (+fb2) DMAed out per core.

All per-core inputs are packed host-side into a single (128, 3727) fp32
tensor loaded by ONE DMA (PE instructions only support a single sync wait,
so first-use deps must collapse to one semaphore).
"""
import os
import sys

for _p in ("/opt/trn_rl_repo", "/root/.axon_site/_ro/trn_rl_repo"):
    if os.path.isdir(_p) and _p not in sys.path:
        sys.path.insert(0, _p)

import numpy as np
import concourse.bass as bass
import concourse.mybir as mybir
from concourse import tile

B, U, T = 16, 64, 128
UAV_DIM, TASK_DIM = 32, 32
E, H, HID = 128, 4, 256
ENC_H = 128
NCORES = 8
BL = B // NCORES          # batches per core
NBLK = U // 4             # 16 u-blocks of 4 us -> N=512 columns each
f32, f16 = mybir.dt.float32, mybir.dt.float16
bf16 = mybir.dt.bfloat16
AF = mybir.ActivationFunctionType
ALU = mybir.AluOpType

# packed constant-tensor column layout (fp32 columns)
_C_UAVT = 0          # (32, 128)
_C_TASKT = 128       # (32, 256)
_C_UW0 = 384         # (32, 128)
_C_TW0 = 512         # (32, 128)
_C_UW1 = 640         # (128, 128)
_C_UW2 = 768
_C_TW1 = 896
_C_TW2 = 1024
_C_ENCB = 1152       # (128, 7): ub0 ub1 ub2 tb0 tb1 tb2 fb2
_C_HQPB = 1159       # (128, 8): col c*4+h
_C_WQK = 1167        # (128, 512): Wq c0 | Wq c1 | Wk c0 | Wk c1
# wz: two 63-col fp16 segments; fw2 chunk c at col c*63+31. The (128x32)
# masked stationary with fw2 at column j is the window [c*63+31-j, +32).
_C_WZ = 1679
_C_TOTAL = 1805

_BUILD_PAT = ["G", "D", "A", "G", "D", "A", "G", "A",
              "G", "D", "A", "G", "G", "D", "A", "G"]

_cache: dict = {}


def _split_multi_waits(nc):
    """Walrus in this toolchain rejects >1 sync wait per engine instruction
    ("Too many sync wait commands"). Hoist extra waits onto preceding
    same-engine NoOps — identical semantics on the in-order engine queues."""
    n_split = 0
    for func in nc.m.functions:
        for bb in func.blocks:
            new = []
            for ins in bb.instructions:
                si = ins.sync_info
                waits = list(si.on_wait) if (si and si.on_wait) else []
                if len(waits) > 1:
                    for k, w in enumerate(waits[:-1]):
                        nop = mybir.InstNoOp(name=f"{ins.name}_hw{k}", ins=[], outs=[])
                        nop.engine = ins.engine
                        nop.sync_info = mybir.SyncInfo(on_wait=[w], on_update=[])
                        new.append(nop)
                        n_split += 1
                    si.on_wait = [waits[-1]]
                new.append(ins)
            bb.instructions = new
    return n_split


def _build_nc():
    nc = bass.Bass()
    packed = nc.dram_tensor("packed", [128, _C_TOTAL], f32, kind="ExternalInput")
    out = nc.dram_tensor("out", [128, 512], f32, kind="ExternalOutput")

    with tile.TileContext(nc) as tc:
        with (
            tc.tile_pool(name="const", bufs=1) as constp,
            tc.tile_pool(name="persist", bufs=1) as persistp,
            tc.tile_pool(name="encw", bufs=2) as encwp,
        ):
            A = constp.tile([128, _C_TOTAL], f32, tag="all")
            nc.gpsimd.dma_start(A[:, :_C_WQK], packed[:, :_C_WQK])
            nc.gpsimd.dma_start(A[:, _C_WQK:], packed[:, _C_WQK:])
            # fp32 -> fp16 conversion on ScalarE: also serves as ScalarE's
            # first touch of the DMA'd tile, so later ACT instructions never
            # pair a DMA-sem wait with an engine-sem wait (ISA wait-slot
            # limits; PE matmuls only support a single wait).
            # wz holds fw2 chunk c in column c*63+31; the (128x32) masked
            # stationary with fw2 at column j is the window [c*63+31-j, +32).
            act_touch = constp.tile([128, 1], f32, tag="acttouch")
            nc.scalar.copy(act_touch[:], A[:, 0:1])
            sb_wz = constp.tile([128, 126], f16, tag="wz")
            nc.scalar.copy(sb_wz[:], A[:, _C_WZ:_C_WZ + 126])
            # VectorE first touch of the DMA'd tile (same wait-slot reason).
            dve_touch = constp.tile([128, 1], f32, tag="dvetouch")
            nc.vector.tensor_copy(dve_touch[:], A[:, 0:1])
            # fp16 projection weights: fp32 moving operands stream at half
            # rate through the PE array, so the base matmuls run fp16.
            sb_wqk16 = constp.tile([128, 512], f16, tag="wqk16")
            nc.scalar.copy(sb_wqk16[:], A[:, _C_WQK:_C_WQK + 512])

            enc_w = {
                "uw0": A[0:32, _C_UW0:_C_UW0 + 128],
                "tw0": A[0:32, _C_TW0:_C_TW0 + 128],
                "uw1": A[:, _C_UW1:_C_UW1 + 128],
                "uw2": A[:, _C_UW2:_C_UW2 + 128],
                "tw1": A[:, _C_TW1:_C_TW1 + 128],
                "tw2": A[:, _C_TW2:_C_TW2 + 128],
            }

            def encb_col(i):
                return A[:, _C_ENCB + i:_C_ENCB + i + 1]

            # ---- pools for the whole kernel (8 PSUM banks exactly:
            #      encoder 1 + base 3 + logits 4) ----
            with (
                tc.tile_pool(name="bsbp", bufs=2) as bsbp,
                tc.tile_pool(name="relup", bufs=4) as relup,
                tc.tile_pool(name="outp", bufs=1) as outp,
                tc.tile_pool(name="bpp", bufs=2, space="PSUM") as psB,
                tc.tile_pool(name="lpp", bufs=1, space="PSUM") as psL,
            ):
                # ---- encoders: chain in (feat x rows) layout ----
                def mlp3(xT, rows, wnames, bcols, tag):
                    cur = xT
                    for li in range(3):
                        ps = psB.tile([128, 512], f32, tag="bp", name=f"ps{tag}{li}")
                        ps = ps[:, :rows]
                        nc.tensor.matmul(ps[:], enc_w[wnames[li]], cur,
                                         start=True, stop=True)
                        if li < 2:
                            nxt = encwp.tile([128, rows], f32, tag=f"{tag}h",
                                             name=f"{tag}h{li}")
                            nc.scalar.activation(nxt[:], ps[:], AF.Relu,
                                                 bias=encb_col(bcols[li]), scale=1.0)
                        else:
                            nxt = persistp.tile([128, rows], f16, tag=f"{tag}T",
                                                name=f"{tag}T")
                            nc.scalar.activation(nxt[:], ps[:], AF.Identity,
                                                 bias=encb_col(bcols[li]), scale=1.0)
                        cur = nxt[:]
                    return cur

                ueT = mlp3(A[0:32, _C_UAVT:_C_UAVT + BL * U], BL * U,
                           ("uw0", "uw1", "uw2"), (0, 1, 2), "ue")
                teT = mlp3(A[0:32, _C_TASKT:_C_TASKT + BL * T], BL * T,
                           ("tw0", "tw1", "tw2"), (3, 4, 5), "te")

                lp = [psL.tile([128, 512], f32, tag=f"lp{g}", name=f"lp{g}")
                      for g in range(4)]
                bi = 0
                NS = 8          # n-blocks per relu slab (FD = NS*512)
                for b in range(BL):
                    for c in range(2):
                        # small projections for this (batch, chunk):
                        # khP[d,t] = (te @ Wk_c)^T,  qP[d,u] = (ue @ Wq_c)^T
                        pk = psB.tile([128, 512], f32, tag="bp", name="pk")
                        nc.tensor.matmul(pk[:, :T],
                                         sb_wqk16[:, 256 + c * 128:256 + (c + 1) * 128],
                                         teT[:, b * T:(b + 1) * T],
                                         start=True, stop=True)
                        pq = psB.tile([128, 512], f32, tag="bp", name="pq")
                        nc.tensor.matmul(pq[:, :U],
                                         sb_wqk16[:, c * 128:(c + 1) * 128],
                                         ueT[:, b * U:(b + 1) * U],
                                         start=True, stop=True)
                        khP = bsbp.tile([128, T], f16, tag="khP", name="khP")
                        nc.scalar.copy(khP[:], pk[:, :T])
                        qP = bsbp.tile([128, U], f32, tag="qP", name="qP")
                        nc.scalar.copy(qP[:], pq[:, :U])
                        for n0 in range(0, NBLK, NS):
                            # base slab: base[d,(u,t)] = khP[d,t] + qP[d,u],
                            # built FD=128 at a time (u-specific bias), split
                            # between ScalarE and VectorE.
                            bsb = bsbp.tile([128, NS * 512], f16, tag="bsb",
                                            name="bsb")
                            for dn in range(NS):
                                for du in range(4):
                                    u = 4 * (n0 + dn) + du
                                    dst = bsb[:, dn * 512 + du * 128:
                                              dn * 512 + (du + 1) * 128]
                                    if bi % 20 >= 13:
                                        nc.vector.tensor_scalar(
                                            dst, khP[:], qP[:, u:u + 1], None,
                                            ALU.add)
                                    else:
                                        nc.scalar.activation(
                                            dst, khP[:], AF.Identity,
                                            bias=qP[:, u:u + 1], scale=1.0)
                                    bi += 1
                            for hp in range(2):
                                # heads hp and hp+2 land in different PE
                                # column groups (strips 2b, 2b+1): interleave
                                # their matmuls so the streams run
                                # concurrently in the array.
                                rts = {}
                                for h in (hp, hp + 2):
                                    rt = relup.tile([128, NS * 512], f16,
                                                    tag="rt", name="rt")
                                    bias_ap = A[:, _C_HQPB + c * 4 + h:
                                                _C_HQPB + c * 4 + h + 1]
                                    nc.vector.tensor_scalar(
                                        rt[:], bsb[:], bias_ap, 0.0,
                                        ALU.add, ALU.max)
                                    rts[h] = rt
                                for dn in range(NS):
                                    n = n0 + dn
                                    for h in (hp, hp + 2):
                                        p_ = (b * H + h) * NBLK + n
                                        g, j = p_ // 32, p_ % 32
                                        first = (c == 0 and n == 0
                                                 and h % 2 == 0)
                                        last = (c == 1 and n == NBLK - 1
                                                and h % 2 == 1)
                                        nc.tensor.matmul(
                                            lp[g][32 * g:32 * g + 32, :],
                                            sb_wz[:, c * 63 + 31 - j:
                                                  c * 63 + 63 - j],
                                            rts[h][:, dn * 512:(dn + 1) * 512],
                                            start=first, stop=last,
                                            tile_position=(0, 32 * g))

                sb_out = outp.tile([128, 512], f32, tag="sbout", name="sbout")
                for g in range(4):
                    nc.vector.tensor_scalar(
                        sb_out[32 * g:32 * g + 32, :],
                        lp[g][32 * g:32 * g + 32, :],
                        A[32 * g:32 * g + 32, _C_ENCB + 6:_C_ENCB + 7],
                        None, ALU.add)
                nc.sync.dma_start(out[:], sb_out[:])
    return nc


def _prep_inputs(uav_feat, task_feat, uw0, ub0, uw1, ub1, uw2, ub2,
                 tw0, tb0, tw1, tb1, tw2, tb2, head_q, fw1, fb1, fw2, fb2):
    f = np.float32
    uav = np.asarray(uav_feat, f)
    task = np.asarray(task_feat, f)
    fw1 = np.asarray(fw1, f)
    fw2 = np.asarray(fw2, f)
    Wq, Wk = fw1[:E], fw1[E:]

    base = np.zeros((128, _C_TOTAL), f)
    base[0:32, _C_UW0:_C_UW0 + 128] = np.asarray(uw0, f)
    base[0:32, _C_TW0:_C_TW0 + 128] = np.asarray(tw0, f)
    base[:, _C_UW1:_C_UW1 + 128] = np.asarray(uw1, f)
    base[:, _C_UW2:_C_UW2 + 128] = np.asarray(uw2, f)
    base[:, _C_TW1:_C_TW1 + 128] = np.asarray(tw1, f)
    base[:, _C_TW2:_C_TW2 + 128] = np.asarray(tw2, f)
    for i, v in enumerate((ub0, ub1, ub2, tb0, tb1, tb2)):
        base[:, _C_ENCB + i] = np.asarray(v, f)
    base[:, _C_ENCB + 6] = np.asarray(fb2, f)[0]
    hq = np.asarray(head_q, f) @ Wq + np.asarray(fb1, f)  # (H, HID)
    for c in range(2):
        for h in range(H):
            base[:, _C_HQPB + c * 4 + h] = hq[h, c * 128:(c + 1) * 128]
    base[:, _C_WQK:_C_WQK + 256] = Wq
    base[:, _C_WQK + 256:_C_WQK + 512] = Wk
    for c in range(2):
        base[:, _C_WZ + c * 63 + 31] = fw2[c * 128:(c + 1) * 128, 0]

    in_maps = []
    for k in range(NCORES):
        b0 = k * BL
        pk = base.copy()
        pk[0:32, _C_UAVT:_C_UAVT + BL * U] = \
            uav[b0:b0 + BL].reshape(BL * U, UAV_DIM).T
        pk[0:32, _C_TASKT:_C_TASKT + BL * T] = \
            task[b0:b0 + BL].reshape(BL * T, TASK_DIM).T
        in_maps.append({"packed": pk})
    return in_maps


def _gather(results):
    outs = []
    for k in range(NCORES):
        r = np.asarray(results[k]["out"], np.float32)  # (128, 512)
        outs.append(r.reshape(BL, H, NBLK, 4, T).reshape(BL, H, U, T))
    return np.concatenate(outs, axis=0)


def kernel(**inputs) -> np.ndarray:
    if "nc" not in _cache:
        _cache["nc"] = _build_nc()
    nc = _cache["nc"]
    in_maps = _prep_inputs(**inputs)
    if os.environ.get("BASS_KERNEL_SIM"):
        # CoreSim can't digest the hand-inserted wait-splitting NoOps; it
        # enforces the multi-wait semantics natively, so run unsplit.
        from concourse.bass_interp import CoreSim
        results = []
        for k in range(NCORES):
            sim = CoreSim(nc)
            for name, arr in in_maps[k].items():
                sim.tensor(name)[:] = arr
            sim.simulate()
            results.append({"out": np.array(sim.tensor("out"))})
    else:
        from concourse.bass_utils import run_bass_kernel_spmd
        if not _cache.get("split"):
            _split_multi_waits(nc)
            _cache["split"] = True
        results = run_bass_kernel_spmd(nc, in_maps, list(range(NCORES))).results
    return _gather(results)


# revision 21
# speedup vs baseline: 1.0935x; 1.0181x over previous
"""Trainium2 Bass kernel for nn_MultiHeadHighLevelAllocator.

Math (reference):
    ue = MLP3(uav_feat)                            # (B,U,E)
    te = MLP3(task_feat)                           # (B,T,E)
    q  = ue[:,None,:,:] + head_q[None,:,None,:]    # (B,H,U,E)
    logits[b,h,u,t] = relu(q[b,h,u]@Wq + te[b,t]@Wk + fb1) @ fw2 + fb2

Key decomposition: by linearity of the projections,
    pre[b,h,u,t,:] = base[b,u,t,:] + hqP[h,:]
where base[b,u,t,:] = ue[b,u]@Wq + te[b,t]@Wk  (outer sum, H-independent)
and   hqP[h,:] = head_q[h]@Wq + fb1.

Per-core (data parallel over B, 2 batches/core):
  1. Encoders on TensorE in transposed layout (feat x rows), ScalarE ReLU+bias.
  2. base tiles (128d x 512) produced by two accumulating matmuls into PSUM
     (stride-0 broadcast APs replicate ue columns over t / te block over u).
  3. Per head: ReLU(base + hqP[h]) with per-partition bias -> fp16 tiles
     (VectorE tensor_scalar add+max for 2 heads, ScalarE activation for 2).
  4. Reduction against fw2 via masked-stationary matmuls: a (128x32) fp16
     stationary holding fw2-chunk in column j writes the dot product row to
     PSUM partition 32g+j of strip-g's own bank, accumulating zeros elsewhere;
     16 u-blocks x 2 heads x 2 chunks accumulate per strip bank group.
  5. One (128x512) fp32 result tile (+fb2) DMAed out per core.

All per-core inputs are packed host-side into a single (128, 3727) fp32
tensor loaded by ONE DMA (PE instructions only support a single sync wait,
so first-use deps must collapse to one semaphore).
"""
import os
import sys

for _p in ("/opt/trn_rl_repo", "/root/.axon_site/_ro/trn_rl_repo"):
    if os.path.isdir(_p) and _p not in sys.path:
        sys.path.insert(0, _p)

import numpy as np
import concourse.bass as bass
import concourse.mybir as mybir
from concourse import tile

B, U, T = 16, 64, 128
UAV_DIM, TASK_DIM = 32, 32
E, H, HID = 128, 4, 256
ENC_H = 128
NCORES = 8
BL = B // NCORES          # batches per core
NBLK = U // 4             # 16 u-blocks of 4 us -> N=512 columns each
f32, f16 = mybir.dt.float32, mybir.dt.float16
bf16 = mybir.dt.bfloat16
AF = mybir.ActivationFunctionType
ALU = mybir.AluOpType

# packed constant-tensor column layout (fp32 columns)
_C_UAVT = 0          # (32, 128)
_C_TASKT = 128       # (32, 256)
_C_UW0 = 384         # (32, 128)
_C_TW0 = 512         # (32, 128)
_C_UW1 = 640         # (128, 128)
_C_UW2 = 768
_C_TW1 = 896
_C_TW2 = 1024
_C_ENCB = 1152       # (128, 7): ub0 ub1 ub2 tb0 tb1 tb2 fb2
_C_HQPB = 1159       # (128, 8): col c*4+h
_C_WQK = 1167        # (128, 512): Wq c0 | Wq c1 | Wk c0 | Wk c1
# wz: two 63-col fp16 segments; fw2 chunk c at col c*63+31. The (128x32)
# masked stationary with fw2 at column j is the window [c*63+31-j, +32).
_C_WZ = 1679
_C_TOTAL = 1805

_BUILD_PAT = ["G", "D", "A", "G", "D", "A", "G", "A",
              "G", "D", "A", "G", "G", "D", "A", "G"]

_cache: dict = {}


def _split_multi_waits(nc):
    """Walrus in this toolchain rejects >1 sync wait per engine instruction
    ("Too many sync wait commands"). Hoist extra waits onto preceding
    same-engine NoOps — identical semantics on the in-order engine queues."""
    n_split = 0
    for func in nc.m.functions:
        for bb in func.blocks:
            new = []
            for ins in bb.instructions:
                si = ins.sync_info
                waits = list(si.on_wait) if (si and si.on_wait) else []
                if len(waits) > 1:
                    for k, w in enumerate(waits[:-1]):
                        nop = mybir.InstNoOp(name=f"{ins.name}_hw{k}", ins=[], outs=[])
                        nop.engine = ins.engine
                        nop.sync_info = mybir.SyncInfo(on_wait=[w], on_update=[])
                        new.append(nop)
                        n_split += 1
                    si.on_wait = [waits[-1]]
                new.append(ins)
            bb.instructions = new
    return n_split


def _build_nc():
    nc = bass.Bass()
    packed = nc.dram_tensor("packed", [128, _C_TOTAL], f32, kind="ExternalInput")
    out = nc.dram_tensor("out", [128, 512], f32, kind="ExternalOutput")

    with tile.TileContext(nc) as tc:
        with (
            tc.tile_pool(name="const", bufs=1) as constp,
            tc.tile_pool(name="persist", bufs=1) as persistp,
            tc.tile_pool(name="encw", bufs=2) as encwp,
        ):
            A = constp.tile([128, _C_TOTAL], f32, tag="all")
            nc.gpsimd.dma_start(A[:, :_C_WQK], packed[:, :_C_WQK])
            nc.gpsimd.dma_start(A[:, _C_WQK:], packed[:, _C_WQK:])
            # fp32 -> fp16 conversion on ScalarE: also serves as ScalarE's
            # first touch of the DMA'd tile, so later ACT instructions never
            # pair a DMA-sem wait with an engine-sem wait (ISA wait-slot
            # limits; PE matmuls only support a single wait).
            # wz holds fw2 chunk c in column c*63+31; the (128x32) masked
            # stationary with fw2 at column j is the window [c*63+31-j, +32).
            act_touch = constp.tile([128, 1], f32, tag="acttouch")
            nc.scalar.copy(act_touch[:], A[:, 0:1])
            sb_wz = constp.tile([128, 126], f16, tag="wz")
            nc.scalar.copy(sb_wz[:], A[:, _C_WZ:_C_WZ + 126])
            # VectorE first touch of the DMA'd tile (same wait-slot reason).
            dve_touch = constp.tile([128, 1], f32, tag="dvetouch")
            nc.vector.tensor_copy(dve_touch[:], A[:, 0:1])
            # fp16 projection weights: fp32 moving operands stream at half
            # rate through the PE array, so the base matmuls run fp16.
            sb_wqk16 = constp.tile([128, 512], f16, tag="wqk16")
            nc.scalar.copy(sb_wqk16[:], A[:, _C_WQK:_C_WQK + 512])

            enc_w = {
                "uw0": A[0:32, _C_UW0:_C_UW0 + 128],
                "tw0": A[0:32, _C_TW0:_C_TW0 + 128],
                "uw1": A[:, _C_UW1:_C_UW1 + 128],
                "uw2": A[:, _C_UW2:_C_UW2 + 128],
                "tw1": A[:, _C_TW1:_C_TW1 + 128],
                "tw2": A[:, _C_TW2:_C_TW2 + 128],
            }

            def encb_col(i):
                return A[:, _C_ENCB + i:_C_ENCB + i + 1]

            # ---- pools for the whole kernel (8 PSUM banks exactly:
            #      encoder 1 + base 3 + logits 4) ----
            with (
                tc.tile_pool(name="bsbp", bufs=3) as bsbp,
                tc.tile_pool(name="relup", bufs=6) as relup,
                tc.tile_pool(name="outp", bufs=1) as outp,
                tc.tile_pool(name="bpp", bufs=2, space="PSUM") as psB,
                tc.tile_pool(name="lpp", bufs=1, space="PSUM") as psL,
            ):
                # ---- encoders: chain in (feat x rows) layout ----
                def mlp3(xT, rows, wnames, bcols, tag):
                    cur = xT
                    for li in range(3):
                        ps = psB.tile([128, 512], f32, tag="bp", name=f"ps{tag}{li}")
                        ps = ps[:, :rows]
                        nc.tensor.matmul(ps[:], enc_w[wnames[li]], cur,
                                         start=True, stop=True)
                        if li < 2:
                            nxt = encwp.tile([128, rows], f32, tag=f"{tag}h",
                                             name=f"{tag}h{li}")
                            nc.scalar.activation(nxt[:], ps[:], AF.Relu,
                                                 bias=encb_col(bcols[li]), scale=1.0)
                        else:
                            nxt = persistp.tile([128, rows], f16, tag=f"{tag}T",
                                                name=f"{tag}T")
                            nc.scalar.activation(nxt[:], ps[:], AF.Identity,
                                                 bias=encb_col(bcols[li]), scale=1.0)
                        cur = nxt[:]
                    return cur

                ueT = mlp3(A[0:32, _C_UAVT:_C_UAVT + BL * U], BL * U,
                           ("uw0", "uw1", "uw2"), (0, 1, 2), "ue")
                teT = mlp3(A[0:32, _C_TASKT:_C_TASKT + BL * T], BL * T,
                           ("tw0", "tw1", "tw2"), (3, 4, 5), "te")

                lp = [psL.tile([128, 512], f32, tag=f"lp{g}", name=f"lp{g}")
                      for g in range(4)]
                bi = 0
                for b in range(BL):
                    for c in range(2):
                        # small projections for this (batch, chunk):
                        # khP[d,t] = (te @ Wk_c)^T,  qP[d,u] = (ue @ Wq_c)^T
                        pk = psB.tile([128, 512], f32, tag="bp", name="pk")
                        nc.tensor.matmul(pk[:, :T],
                                         sb_wqk16[:, 256 + c * 128:256 + (c + 1) * 128],
                                         teT[:, b * T:(b + 1) * T],
                                         start=True, stop=True)
                        pq = psB.tile([128, 512], f32, tag="bp", name="pq")
                        nc.tensor.matmul(pq[:, :U],
                                         sb_wqk16[:, c * 128:(c + 1) * 128],
                                         ueT[:, b * U:(b + 1) * U],
                                         start=True, stop=True)
                        khP = bsbp.tile([128, T], f16, tag="khP", name="khP")
                        nc.scalar.copy(khP[:], pk[:, :T])
                        qP = bsbp.tile([128, U], f32, tag="qP", name="qP")
                        nc.scalar.copy(qP[:], pq[:, :U])
                        bc_idx = 2 * b + c
                        if bc_idx == 0:
                            slab_plan = [2, 2, 4, 8]
                        elif bc_idx == 3:
                            slab_plan = [8, 4, 2, 2]
                        else:
                            slab_plan = [8, 8]
                        n0 = 0
                        for NS in slab_plan:
                            # base slab: base[d,(u,t)] = khP[d,t] + qP[d,u],
                            # built FD=128 at a time (u-specific bias), split
                            # between ScalarE and VectorE.
                            bsb = bsbp.tile([128, NS * 512], f16, tag="bsb",
                                            name="bsb")
                            for dn in range(NS):
                                for du in range(4):
                                    u = 4 * (n0 + dn) + du
                                    dst = bsb[:, dn * 512 + du * 128:
                                              dn * 512 + (du + 1) * 128]
                                    if bi % 20 >= 13:
                                        nc.vector.tensor_scalar(
                                            dst, khP[:], qP[:, u:u + 1], None,
                                            ALU.add)
                                    else:
                                        nc.scalar.activation(
                                            dst, khP[:], AF.Identity,
                                            bias=qP[:, u:u + 1], scale=1.0)
                                    bi += 1
                            for hp in range(2):
                                # heads hp and hp+2 land in different PE
                                # column groups (strips 2b, 2b+1): interleave
                                # their matmuls so the streams run
                                # concurrently in the array.
                                rts = {}
                                for h in (hp, hp + 2):
                                    rt = relup.tile([128, NS * 512], f16,
                                                    tag="rt", name="rt")
                                    bias_ap = A[:, _C_HQPB + c * 4 + h:
                                                _C_HQPB + c * 4 + h + 1]
                                    nc.vector.tensor_scalar(
                                        rt[:], bsb[:], bias_ap, 0.0,
                                        ALU.add, ALU.max)
                                    rts[h] = rt
                                for dn in range(NS):
                                    n = n0 + dn
                                    for h in (hp, hp + 2):
                                        p_ = (b * H + h) * NBLK + n
                                        g, j = p_ // 32, p_ % 32
                                        first = (c == 0 and n == 0
                                                 and h % 2 == 0)
                                        last = (c == 1 and n == NBLK - 1
                                                and h % 2 == 1)
                                        nc.tensor.matmul(
                                            lp[g][32 * g:32 * g + 32, :],
                                            sb_wz[:, c * 63 + 31 - j:
                                                  c * 63 + 63 - j],
                                            rts[h][:, dn * 512:(dn + 1) * 512],
                                            start=first, stop=last,
                                            tile_position=(0, 32 * g))
                            n0 += NS

                sb_out = outp.tile([128, 512], f32, tag="sbout", name="sbout")
                for g in range(4):
                    nc.vector.tensor_scalar(
                        sb_out[32 * g:32 * g + 32, :],
                        lp[g][32 * g:32 * g + 32, :],
                        A[32 * g:32 * g + 32, _C_ENCB + 6:_C_ENCB + 7],
                        None, ALU.add)
                nc.sync.dma_start(out[:], sb_out[:])
    return nc


def _prep_inputs(uav_feat, task_feat, uw0, ub0, uw1, ub1, uw2, ub2,
                 tw0, tb0, tw1, tb1, tw2, tb2, head_q, fw1, fb1, fw2, fb2):
    f = np.float32
    uav = np.asarray(uav_feat, f)
    task = np.asarray(task_feat, f)
    fw1 = np.asarray(fw1, f)
    fw2 = np.asarray(fw2, f)
    Wq, Wk = fw1[:E], fw1[E:]

    base = np.zeros((128, _C_TOTAL), f)
    base[0:32, _C_UW0:_C_UW0 + 128] = np.asarray(uw0, f)
    base[0:32, _C_TW0:_C_TW0 + 128] = np.asarray(tw0, f)
    base[:, _C_UW1:_C_UW1 + 128] = np.asarray(uw1, f)
    base[:, _C_UW2:_C_UW2 + 128] = np.asarray(uw2, f)
    base[:, _C_TW1:_C_TW1 + 128] = np.asarray(tw1, f)
    base[:, _C_TW2:_C_TW2 + 128] = np.asarray(tw2, f)
    for i, v in enumerate((ub0, ub1, ub2, tb0, tb1, tb2)):
        base[:, _C_ENCB + i] = np.asarray(v, f)
    base[:, _C_ENCB + 6] = np.asarray(fb2, f)[0]
    hq = np.asarray(head_q, f) @ Wq + np.asarray(fb1, f)  # (H, HID)
    for c in range(2):
        for h in range(H):
            base[:, _C_HQPB + c * 4 + h] = hq[h, c * 128:(c + 1) * 128]
    base[:, _C_WQK:_C_WQK + 256] = Wq
    base[:, _C_WQK + 256:_C_WQK + 512] = Wk
    for c in range(2):
        base[:, _C_WZ + c * 63 + 31] = fw2[c * 128:(c + 1) * 128, 0]

    in_maps = []
    for k in range(NCORES):
        b0 = k * BL
        pk = base.copy()
        pk[0:32, _C_UAVT:_C_UAVT + BL * U] = \
            uav[b0:b0 + BL].reshape(BL * U, UAV_DIM).T
        pk[0:32, _C_TASKT:_C_TASKT + BL * T] = \
            task[b0:b0 + BL].reshape(BL * T, TASK_DIM).T
        in_maps.append({"packed": pk})
    return in_maps


def _gather(results):
    outs = []
    for k in range(NCORES):
        r = np.asarray(results[k]["out"], np.float32)  # (128, 512)
        outs.append(r.reshape(BL, H, NBLK, 4, T).reshape(BL, H, U, T))
    return np.concatenate(outs, axis=0)


def kernel(**inputs) -> np.ndarray:
    if "nc" not in _cache:
        _cache["nc"] = _build_nc()
    nc = _cache["nc"]
    in_maps = _prep_inputs(**inputs)
    if os.environ.get("BASS_KERNEL_SIM"):
        # CoreSim can't digest the hand-inserted wait-splitting NoOps; it
        # enforces the multi-wait semantics natively, so run unsplit.
        from concourse.bass_interp import CoreSim
        results = []
        for k in range(NCORES):
            sim = CoreSim(nc)
            for name, arr in in_maps[k].items():
                sim.tensor(name)[:] = arr
            sim.simulate()
            results.append({"out": np.array(sim.tensor("out"))})
    else:
        from concourse.bass_utils import run_bass_kernel_spmd
        if not _cache.get("split"):
            _split_multi_waits(nc)
            _cache["split"] = True
        results = run_bass_kernel_spmd(nc, in_maps, list(range(NCORES))).results
    return _gather(results)


# revision 22
# speedup vs baseline: 1.0981x; 1.0041x over previous
"""Trainium2 Bass kernel for nn_MultiHeadHighLevelAllocator.

Math (reference):
    ue = MLP3(uav_feat)                            # (B,U,E)
    te = MLP3(task_feat)                           # (B,T,E)
    q  = ue[:,None,:,:] + head_q[None,:,None,:]    # (B,H,U,E)
    logits[b,h,u,t] = relu(q[b,h,u]@Wq + te[b,t]@Wk + fb1) @ fw2 + fb2

Key decomposition: by linearity of the projections,
    pre[b,h,u,t,:] = base[b,u,t,:] + hqP[h,:]
where base[b,u,t,:] = ue[b,u]@Wq + te[b,t]@Wk  (outer sum, H-independent)
and   hqP[h,:] = head_q[h]@Wq + fb1.

Per-core (data parallel over B, 2 batches/core):
  1. Encoders on TensorE in transposed layout (feat x rows), ScalarE ReLU+bias.
  2. base tiles (128d x 512) produced by two accumulating matmuls into PSUM
     (stride-0 broadcast APs replicate ue columns over t / te block over u).
  3. Per head: ReLU(base + hqP[h]) with per-partition bias -> fp16 tiles
     (VectorE tensor_scalar add+max for 2 heads, ScalarE activation for 2).
  4. Reduction against fw2 via masked-stationary matmuls: a (128x32) fp16
     stationary holding fw2-chunk in column j writes the dot product row to
     PSUM partition 32g+j of strip-g's own bank, accumulating zeros elsewhere;
     16 u-blocks x 2 heads x 2 chunks accumulate per strip bank group.
  5. One (128x512) fp32 result tile (+fb2) DMAed out per core.

All per-core inputs are packed host-side into a single (128, 3727) fp32
tensor loaded by ONE DMA (PE instructions only support a single sync wait,
so first-use deps must collapse to one semaphore).
"""
import os
import sys

for _p in ("/opt/trn_rl_repo", "/root/.axon_site/_ro/trn_rl_repo"):
    if os.path.isdir(_p) and _p not in sys.path:
        sys.path.insert(0, _p)

import numpy as np
import concourse.bass as bass
import concourse.mybir as mybir
from concourse import tile

B, U, T = 16, 64, 128
UAV_DIM, TASK_DIM = 32, 32
E, H, HID = 128, 4, 256
ENC_H = 128
NCORES = 8
BL = B // NCORES          # batches per core
NBLK = U // 4             # 16 u-blocks of 4 us -> N=512 columns each
f32, f16 = mybir.dt.float32, mybir.dt.float16
bf16 = mybir.dt.bfloat16
AF = mybir.ActivationFunctionType
ALU = mybir.AluOpType

# packed constant-tensor column layout (fp32 columns)
_C_UAVT = 0          # (32, 128)
_C_TASKT = 128       # (32, 256)
_C_UW0 = 384         # (32, 128)
_C_TW0 = 512         # (32, 128)
_C_UW1 = 640         # (128, 128)
_C_UW2 = 768
_C_TW1 = 896
_C_TW2 = 1024
_C_ENCB = 1152       # (128, 7): ub0 ub1 ub2 tb0 tb1 tb2 fb2
_C_HQPB = 1159       # (128, 8): col c*4+h
_C_WQK = 1167        # (128, 512): Wq c0 | Wq c1 | Wk c0 | Wk c1
# wz: two 63-col fp16 segments; fw2 chunk c at col c*63+31. The (128x32)
# masked stationary with fw2 at column j is the window [c*63+31-j, +32).
_C_WZ = 1679
_C_TOTAL = 1805

_BUILD_PAT = ["G", "D", "A", "G", "D", "A", "G", "A",
              "G", "D", "A", "G", "G", "D", "A", "G"]

_cache: dict = {}


def _split_multi_waits(nc):
    """Walrus in this toolchain rejects >1 sync wait per engine instruction
    ("Too many sync wait commands"). Hoist extra waits onto preceding
    same-engine NoOps — identical semantics on the in-order engine queues."""
    n_split = 0
    for func in nc.m.functions:
        for bb in func.blocks:
            new = []
            for ins in bb.instructions:
                si = ins.sync_info
                waits = list(si.on_wait) if (si and si.on_wait) else []
                if len(waits) > 1:
                    for k, w in enumerate(waits[:-1]):
                        nop = mybir.InstNoOp(name=f"{ins.name}_hw{k}", ins=[], outs=[])
                        nop.engine = ins.engine
                        nop.sync_info = mybir.SyncInfo(on_wait=[w], on_update=[])
                        new.append(nop)
                        n_split += 1
                    si.on_wait = [waits[-1]]
                new.append(ins)
            bb.instructions = new
    return n_split


def _build_nc():
    nc = bass.Bass()
    packed = nc.dram_tensor("packed", [128, _C_TOTAL], f32, kind="ExternalInput")
    out = nc.dram_tensor("out", [128, 512], f32, kind="ExternalOutput")

    with tile.TileContext(nc) as tc:
        with (
            tc.tile_pool(name="const", bufs=1) as constp,
            tc.tile_pool(name="persist", bufs=1) as persistp,
            tc.tile_pool(name="encw", bufs=2) as encwp,
        ):
            A = constp.tile([128, _C_TOTAL], f32, tag="all")
            nc.sync.dma_start(A[:, :_C_WQK], packed[:, :_C_WQK])
            nc.sync.dma_start(A[:, _C_WQK:], packed[:, _C_WQK:])
            # fp32 -> fp16 conversion on ScalarE: also serves as ScalarE's
            # first touch of the DMA'd tile, so later ACT instructions never
            # pair a DMA-sem wait with an engine-sem wait (ISA wait-slot
            # limits; PE matmuls only support a single wait).
            # wz holds fw2 chunk c in column c*63+31; the (128x32) masked
            # stationary with fw2 at column j is the window [c*63+31-j, +32).
            act_touch = constp.tile([128, 1], f32, tag="acttouch")
            nc.scalar.copy(act_touch[:], A[:, 0:1])
            sb_wz = constp.tile([128, 126], f16, tag="wz")
            nc.scalar.copy(sb_wz[:], A[:, _C_WZ:_C_WZ + 126])
            # VectorE first touch of the DMA'd tile (same wait-slot reason).
            dve_touch = constp.tile([128, 1], f32, tag="dvetouch")
            nc.vector.tensor_copy(dve_touch[:], A[:, 0:1])
            # fp16 projection weights: fp32 moving operands stream at half
            # rate through the PE array, so the base matmuls run fp16.
            sb_wqk16 = constp.tile([128, 512], f16, tag="wqk16")
            nc.scalar.copy(sb_wqk16[:], A[:, _C_WQK:_C_WQK + 512])

            enc_w = {
                "uw0": A[0:32, _C_UW0:_C_UW0 + 128],
                "tw0": A[0:32, _C_TW0:_C_TW0 + 128],
                "uw1": A[:, _C_UW1:_C_UW1 + 128],
                "uw2": A[:, _C_UW2:_C_UW2 + 128],
                "tw1": A[:, _C_TW1:_C_TW1 + 128],
                "tw2": A[:, _C_TW2:_C_TW2 + 128],
            }

            def encb_col(i):
                return A[:, _C_ENCB + i:_C_ENCB + i + 1]

            # ---- pools for the whole kernel (8 PSUM banks exactly:
            #      encoder 1 + base 3 + logits 4) ----
            with (
                tc.tile_pool(name="bsbp", bufs=3) as bsbp,
                tc.tile_pool(name="relup", bufs=6) as relup,
                tc.tile_pool(name="outp", bufs=1) as outp,
                tc.tile_pool(name="bpp", bufs=2, space="PSUM") as psB,
                tc.tile_pool(name="lpp", bufs=1, space="PSUM") as psL,
            ):
                # ---- encoders: chain in (feat x rows) layout ----
                def mlp3(xT, rows, wnames, bcols, tag):
                    cur = xT
                    for li in range(3):
                        ps = psB.tile([128, 512], f32, tag="bp", name=f"ps{tag}{li}")
                        ps = ps[:, :rows]
                        nc.tensor.matmul(ps[:], enc_w[wnames[li]], cur,
                                         start=True, stop=True)
                        if li < 2:
                            nxt = encwp.tile([128, rows], f32, tag=f"{tag}h",
                                             name=f"{tag}h{li}")
                            nc.scalar.activation(nxt[:], ps[:], AF.Relu,
                                                 bias=encb_col(bcols[li]), scale=1.0)
                        else:
                            nxt = persistp.tile([128, rows], f16, tag=f"{tag}T",
                                                name=f"{tag}T")
                            nc.scalar.activation(nxt[:], ps[:], AF.Identity,
                                                 bias=encb_col(bcols[li]), scale=1.0)
                        cur = nxt[:]
                    return cur

                ueT = mlp3(A[0:32, _C_UAVT:_C_UAVT + BL * U], BL * U,
                           ("uw0", "uw1", "uw2"), (0, 1, 2), "ue")
                teT = mlp3(A[0:32, _C_TASKT:_C_TASKT + BL * T], BL * T,
                           ("tw0", "tw1", "tw2"), (3, 4, 5), "te")

                lp = [psL.tile([128, 512], f32, tag=f"lp{g}", name=f"lp{g}")
                      for g in range(4)]
                bi = 0
                for b in range(BL):
                    for c in range(2):
                        # small projections for this (batch, chunk):
                        # khP[d,t] = (te @ Wk_c)^T,  qP[d,u] = (ue @ Wq_c)^T
                        pk = psB.tile([128, 512], f32, tag="bp", name="pk")
                        nc.tensor.matmul(pk[:, :T],
                                         sb_wqk16[:, 256 + c * 128:256 + (c + 1) * 128],
                                         teT[:, b * T:(b + 1) * T],
                                         start=True, stop=True)
                        pq = psB.tile([128, 512], f32, tag="bp", name="pq")
                        nc.tensor.matmul(pq[:, :U],
                                         sb_wqk16[:, c * 128:(c + 1) * 128],
                                         ueT[:, b * U:(b + 1) * U],
                                         start=True, stop=True)
                        khP = bsbp.tile([128, T], f16, tag="khP", name="khP")
                        nc.scalar.copy(khP[:], pk[:, :T])
                        qP = bsbp.tile([128, U], f32, tag="qP", name="qP")
                        nc.scalar.copy(qP[:], pq[:, :U])
                        bc_idx = 2 * b + c
                        if bc_idx == 0:
                            slab_plan = [2, 2, 4, 8]
                        elif bc_idx == 3:
                            slab_plan = [8, 4, 2, 2]
                        else:
                            slab_plan = [8, 8]
                        n0 = 0
                        for NS in slab_plan:
                            # base slab: base[d,(u,t)] = khP[d,t] + qP[d,u],
                            # built FD=128 at a time (u-specific bias), split
                            # between ScalarE and VectorE.
                            bsb = bsbp.tile([128, NS * 512], f16, tag="bsb",
                                            name="bsb")
                            for dn in range(NS):
                                for du in range(4):
                                    u = 4 * (n0 + dn) + du
                                    dst = bsb[:, dn * 512 + du * 128:
                                              dn * 512 + (du + 1) * 128]
                                    if bi % 20 >= 13:
                                        nc.vector.tensor_scalar(
                                            dst, khP[:], qP[:, u:u + 1], None,
                                            ALU.add)
                                    else:
                                        nc.scalar.activation(
                                            dst, khP[:], AF.Identity,
                                            bias=qP[:, u:u + 1], scale=1.0)
                                    bi += 1
                            for hp in range(2):
                                # heads hp and hp+2 land in different PE
                                # column groups (strips 2b, 2b+1): interleave
                                # their matmuls so the streams run
                                # concurrently in the array.
                                rts = {}
                                for h in (hp, hp + 2):
                                    rt = relup.tile([128, NS * 512], f16,
                                                    tag="rt", name="rt")
                                    bias_ap = A[:, _C_HQPB + c * 4 + h:
                                                _C_HQPB + c * 4 + h + 1]
                                    nc.vector.tensor_scalar(
                                        rt[:], bsb[:], bias_ap, 0.0,
                                        ALU.add, ALU.max)
                                    rts[h] = rt
                                for dn in range(NS):
                                    n = n0 + dn
                                    for h in (hp, hp + 2):
                                        p_ = (b * H + h) * NBLK + n
                                        g, j = p_ // 32, p_ % 32
                                        first = (c == 0 and n == 0
                                                 and h % 2 == 0)
                                        last = (c == 1 and n == NBLK - 1
                                                and h % 2 == 1)
                                        nc.tensor.matmul(
                                            lp[g][32 * g:32 * g + 32, :],
                                            sb_wz[:, c * 63 + 31 - j:
                                                  c * 63 + 63 - j],
                                            rts[h][:, dn * 512:(dn + 1) * 512],
                                            start=first, stop=last,
                                            tile_position=(0, 32 * g))
                            n0 += NS

                sb_out = outp.tile([128, 512], f32, tag="sbout", name="sbout")
                for g in range(4):
                    nc.vector.tensor_scalar(
                        sb_out[32 * g:32 * g + 32, :],
                        lp[g][32 * g:32 * g + 32, :],
                        A[32 * g:32 * g + 32, _C_ENCB + 6:_C_ENCB + 7],
                        None, ALU.add)
                nc.sync.dma_start(out[:], sb_out[:])
    return nc


def _prep_inputs(uav_feat, task_feat, uw0, ub0, uw1, ub1, uw2, ub2,
                 tw0, tb0, tw1, tb1, tw2, tb2, head_q, fw1, fb1, fw2, fb2):
    f = np.float32
    uav = np.asarray(uav_feat, f)
    task = np.asarray(task_feat, f)
    fw1 = np.asarray(fw1, f)
    fw2 = np.asarray(fw2, f)
    Wq, Wk = fw1[:E], fw1[E:]

    base = np.zeros((128, _C_TOTAL), f)
    base[0:32, _C_UW0:_C_UW0 + 128] = np.asarray(uw0, f)
    base[0:32, _C_TW0:_C_TW0 + 128] = np.asarray(tw0, f)
    base[:, _C_UW1:_C_UW1 + 128] = np.asarray(uw1, f)
    base[:, _C_UW2:_C_UW2 + 128] = np.asarray(uw2, f)
    base[:, _C_TW1:_C_TW1 + 128] = np.asarray(tw1, f)
    base[:, _C_TW2:_C_TW2 + 128] = np.asarray(tw2, f)
    for i, v in enumerate((ub0, ub1, ub2, tb0, tb1, tb2)):
        base[:, _C_ENCB + i] = np.asarray(v, f)
    base[:, _C_ENCB + 6] = np.asarray(fb2, f)[0]
    hq = np.asarray(head_q, f) @ Wq + np.asarray(fb1, f)  # (H, HID)
    for c in range(2):
        for h in range(H):
            base[:, _C_HQPB + c * 4 + h] = hq[h, c * 128:(c + 1) * 128]
    base[:, _C_WQK:_C_WQK + 256] = Wq
    base[:, _C_WQK + 256:_C_WQK + 512] = Wk
    for c in range(2):
        base[:, _C_WZ + c * 63 + 31] = fw2[c * 128:(c + 1) * 128, 0]

    in_maps = []
    for k in range(NCORES):
        b0 = k * BL
        pk = base.copy()
        pk[0:32, _C_UAVT:_C_UAVT + BL * U] = \
            uav[b0:b0 + BL].reshape(BL * U, UAV_DIM).T
        pk[0:32, _C_TASKT:_C_TASKT + BL * T] = \
            task[b0:b0 + BL].reshape(BL * T, TASK_DIM).T
        in_maps.append({"packed": pk})
    return in_maps


def _gather(results):
    outs = []
    for k in range(NCORES):
        r = np.asarray(results[k]["out"], np.float32)  # (128, 512)
        outs.append(r.reshape(BL, H, NBLK, 4, T).reshape(BL, H, U, T))
    return np.concatenate(outs, axis=0)


def kernel(**inputs) -> np.ndarray:
    if "nc" not in _cache:
        _cache["nc"] = _build_nc()
    nc = _cache["nc"]
    in_maps = _prep_inputs(**inputs)
    if os.environ.get("BASS_KERNEL_SIM"):
        # CoreSim can't digest the hand-inserted wait-splitting NoOps; it
        # enforces the multi-wait semantics natively, so run unsplit.
        from concourse.bass_interp import CoreSim
        results = []
        for k in range(NCORES):
            sim = CoreSim(nc)
            for name, arr in in_maps[k].items():
                sim.tensor(name)[:] = arr
            sim.simulate()
            results.append({"out": np.array(sim.tensor("out"))})
    else:
        from concourse.bass_utils import run_bass_kernel_spmd
        if not _cache.get("split"):
            _split_multi_waits(nc)
            _cache["split"] = True
        results = run_bass_kernel_spmd(nc, in_maps, list(range(NCORES))).results
    return _gather(results)


# revision 23
# speedup vs baseline: 1.1819x; 1.0763x over previous
"""Trainium2 Bass kernel for nn_MultiHeadHighLevelAllocator.

Math (reference):
    ue = MLP3(uav_feat)                            # (B,U,E)
    te = MLP3(task_feat)                           # (B,T,E)
    q  = ue[:,None,:,:] + head_q[None,:,None,:]    # (B,H,U,E)
    logits[b,h,u,t] = relu(q[b,h,u]@Wq + te[b,t]@Wk + fb1) @ fw2 + fb2

Key decomposition: by linearity of the projections,
    pre[b,h,u,t,:] = base[b,u,t,:] + hqP[h,:]
where base[b,u,t,:] = ue[b,u]@Wq + te[b,t]@Wk  (outer sum, H-independent)
and   hqP[h,:] = head_q[h]@Wq + fb1.

Per-core (data parallel over B, 2 batches/core):
  1. Encoders on TensorE in transposed layout (feat x rows), ScalarE ReLU+bias.
  2. base tiles (128d x 512) produced by two accumulating matmuls into PSUM
     (stride-0 broadcast APs replicate ue columns over t / te block over u).
  3. Per head: ReLU(base + hqP[h]) with per-partition bias -> fp16 tiles
     (VectorE tensor_scalar add+max for 2 heads, ScalarE activation for 2).
  4. Reduction against fw2 via masked-stationary matmuls: a (128x32) fp16
     stationary holding fw2-chunk in column j writes the dot product row to
     PSUM partition 32g+j of strip-g's own bank, accumulating zeros elsewhere;
     16 u-blocks x 2 heads x 2 chunks accumulate per strip bank group.
  5. One (128x512) fp32 result tile (+fb2) DMAed out per core.

All per-core inputs are packed host-side into a single (128, 3727) fp32
tensor loaded by ONE DMA (PE instructions only support a single sync wait,
so first-use deps must collapse to one semaphore).
"""
import os
import sys

for _p in ("/opt/trn_rl_repo", "/root/.axon_site/_ro/trn_rl_repo"):
    if os.path.isdir(_p) and _p not in sys.path:
        sys.path.insert(0, _p)

import numpy as np
import concourse.bass as bass
import concourse.mybir as mybir
from concourse import tile

B, U, T = 16, 64, 128
UAV_DIM, TASK_DIM = 32, 32
E, H, HID = 128, 4, 256
ENC_H = 128
NCORES = 8
BL = B // NCORES          # batches per core
NBLK = U // 4             # 16 u-blocks of 4 us -> N=512 columns each
f32, f16 = mybir.dt.float32, mybir.dt.float16
bf16 = mybir.dt.bfloat16
AF = mybir.ActivationFunctionType
ALU = mybir.AluOpType

# packed constant-tensor column layout (fp32 columns)
_C_UAVT = 0          # (32, 128)
_C_TASKT = 128       # (32, 256)
_C_UW0 = 384         # (32, 128)
_C_TW0 = 512         # (32, 128)
_C_UW1 = 640         # (128, 128)
_C_UW2 = 768
_C_TW1 = 896
_C_TW2 = 1024
_C_ENCB = 1152       # (128, 7): ub0 ub1 ub2 tb0 tb1 tb2 fb2
_C_HQPB = 1159       # (128, 8): col c*4+h
_C_WQK = 1167        # (128, 512): Wq c0 | Wq c1 | Wk c0 | Wk c1
# wz: two 63-col fp16 segments; fw2 chunk c at col c*63+31. The (128x32)
# masked stationary with fw2 at column j is the window [c*63+31-j, +32).
_C_WZ = 1679
_C_TOTAL = 1805

_BUILD_PAT = ["G", "D", "A", "G", "D", "A", "G", "A",
              "G", "D", "A", "G", "G", "D", "A", "G"]

_cache: dict = {}


def _split_multi_waits(nc):
    """Walrus in this toolchain rejects >1 sync wait per engine instruction
    ("Too many sync wait commands"). Hoist extra waits onto preceding
    same-engine NoOps — identical semantics on the in-order engine queues."""
    n_split = 0
    for func in nc.m.functions:
        for bb in func.blocks:
            new = []
            for ins in bb.instructions:
                si = ins.sync_info
                waits = list(si.on_wait) if (si and si.on_wait) else []
                if len(waits) > 1:
                    for k, w in enumerate(waits[:-1]):
                        nop = mybir.InstNoOp(name=f"{ins.name}_hw{k}", ins=[], outs=[])
                        nop.engine = ins.engine
                        nop.sync_info = mybir.SyncInfo(on_wait=[w], on_update=[])
                        new.append(nop)
                        n_split += 1
                    si.on_wait = [waits[-1]]
                new.append(ins)
            bb.instructions = new
    return n_split


def _build_nc():
    nc = bass.Bass()
    packed = nc.dram_tensor("packed", [128, _C_TOTAL], f32, kind="ExternalInput")
    out = nc.dram_tensor("out", [128, 512], f32, kind="ExternalOutput")

    with tile.TileContext(nc) as tc:
        with (
            tc.tile_pool(name="const", bufs=1) as constp,
            tc.tile_pool(name="persist", bufs=1) as persistp,
            tc.tile_pool(name="encw", bufs=2) as encwp,
        ):
            A = constp.tile([128, _C_TOTAL], f32, tag="all")
            # phase-ordered loads so the encoders start as early as possible
            nc.sync.dma_start(A[:, :640], packed[:, :640])           # L1 inputs
            nc.sync.dma_start(A[:, 1152:_C_WQK], packed[:, 1152:_C_WQK])  # biases
            nc.sync.dma_start(A[:, 640:1152], packed[:, 640:1152])   # L2/L3 w
            nc.sync.dma_start(A[:, _C_WQK:], packed[:, _C_WQK:])     # wqk+wz
            # fp32 -> fp16 conversion on ScalarE: also serves as ScalarE's
            # first touch of the DMA'd tile, so later ACT instructions never
            # pair a DMA-sem wait with an engine-sem wait (ISA wait-slot
            # limits; PE matmuls only support a single wait).
            # wz holds fw2 chunk c in column c*63+31; the (128x32) masked
            # stationary with fw2 at column j is the window [c*63+31-j, +32).
            act_touch = constp.tile([128, 1], f32, tag="acttouch")
            nc.scalar.copy(act_touch[:], A[:, 0:1])
            sb_wz = constp.tile([128, 126], f16, tag="wz")
            nc.scalar.copy(sb_wz[:], A[:, _C_WZ:_C_WZ + 126])
            # VectorE first touch of the DMA'd tile (same wait-slot reason).
            dve_touch = constp.tile([128, 1], f32, tag="dvetouch")
            nc.vector.tensor_copy(dve_touch[:], A[:, 0:1])
            # fp16 projection weights: fp32 moving operands stream at half
            # rate through the PE array, so the base matmuls run fp16.
            sb_wqk16 = constp.tile([128, 512], f16, tag="wqk16")
            nc.scalar.copy(sb_wqk16[:], A[:, _C_WQK:_C_WQK + 512])

            enc_w = {
                "uw0": A[0:32, _C_UW0:_C_UW0 + 128],
                "tw0": A[0:32, _C_TW0:_C_TW0 + 128],
                "uw1": A[:, _C_UW1:_C_UW1 + 128],
                "uw2": A[:, _C_UW2:_C_UW2 + 128],
                "tw1": A[:, _C_TW1:_C_TW1 + 128],
                "tw2": A[:, _C_TW2:_C_TW2 + 128],
            }

            def encb_col(i):
                return A[:, _C_ENCB + i:_C_ENCB + i + 1]

            # ---- pools for the whole kernel (8 PSUM banks exactly:
            #      encoder 1 + base 3 + logits 4) ----
            with (
                tc.tile_pool(name="bsbp", bufs=3) as bsbp,
                tc.tile_pool(name="relup", bufs=6) as relup,
                tc.tile_pool(name="outp", bufs=1) as outp,
                tc.tile_pool(name="bpp", bufs=4, space="PSUM") as psB,
                tc.tile_pool(name="lpp", bufs=1, space="PSUM") as psL,
            ):
                # ---- encoders: chain in (feat x rows) layout ----
                def mlp3(xT, rows, wnames, bcols, tag):
                    cur = xT
                    for li in range(3):
                        ps = psB.tile([128, 512], f32, tag="bp", name=f"ps{tag}{li}")
                        ps = ps[:, :rows]
                        nc.tensor.matmul(ps[:], enc_w[wnames[li]], cur,
                                         start=True, stop=True)
                        if li < 2:
                            nxt = encwp.tile([128, rows], f32, tag=f"{tag}h",
                                             name=f"{tag}h{li}")
                            nc.scalar.activation(nxt[:], ps[:], AF.Relu,
                                                 bias=encb_col(bcols[li]), scale=1.0)
                        else:
                            nxt = persistp.tile([128, rows], f16, tag=f"{tag}T",
                                                name=f"{tag}T")
                            nc.scalar.activation(nxt[:], ps[:], AF.Identity,
                                                 bias=encb_col(bcols[li]), scale=1.0)
                        cur = nxt[:]
                    return cur

                ueT = mlp3(A[0:32, _C_UAVT:_C_UAVT + BL * U], BL * U,
                           ("uw0", "uw1", "uw2"), (0, 1, 2), "ue")
                teT = mlp3(A[0:32, _C_TASKT:_C_TASKT + BL * T], BL * T,
                           ("tw0", "tw1", "tw2"), (3, 4, 5), "te")

                lp = [psL.tile([128, 512], f32, tag=f"lp{g}", name=f"lp{g}")
                      for g in range(4)]
                bi = 0
                for b in range(BL):
                    for c in range(2):
                        # small projections for this (batch, chunk):
                        # khP[d,t] = (te @ Wk_c)^T,  qP[d,u] = (ue @ Wq_c)^T
                        pk = psB.tile([128, 512], f32, tag="bp", name="pk")
                        nc.tensor.matmul(pk[:, :T],
                                         sb_wqk16[:, 256 + c * 128:256 + (c + 1) * 128],
                                         teT[:, b * T:(b + 1) * T],
                                         start=True, stop=True)
                        pq = psB.tile([128, 512], f32, tag="bp", name="pq")
                        nc.tensor.matmul(pq[:, :U],
                                         sb_wqk16[:, c * 128:(c + 1) * 128],
                                         ueT[:, b * U:(b + 1) * U],
                                         start=True, stop=True)
                        khP = bsbp.tile([128, T], f16, tag="khP", name="khP")
                        nc.scalar.copy(khP[:], pk[:, :T])
                        qP = bsbp.tile([128, U], f32, tag="qP", name="qP")
                        nc.scalar.copy(qP[:], pq[:, :U])
                        bc_idx = 2 * b + c
                        if bc_idx == 0:
                            slab_plan = [2, 2, 4, 8]
                        elif bc_idx == 3:
                            slab_plan = [8, 4, 2, 2]
                        else:
                            slab_plan = [8, 8]
                        n0 = 0
                        for NS in slab_plan:
                            # base slab: base[d,(u,t)] = khP[d,t] + qP[d,u],
                            # built FD=128 at a time (u-specific bias), split
                            # between ScalarE and VectorE.
                            bsb = bsbp.tile([128, NS * 512], f16, tag="bsb",
                                            name="bsb")
                            for dn in range(NS):
                                for du in range(4):
                                    u = 4 * (n0 + dn) + du
                                    dst = bsb[:, dn * 512 + du * 128:
                                              dn * 512 + (du + 1) * 128]
                                    if bi % 20 >= 13:
                                        nc.vector.tensor_scalar(
                                            dst, khP[:], qP[:, u:u + 1], None,
                                            ALU.add)
                                    else:
                                        nc.scalar.activation(
                                            dst, khP[:], AF.Identity,
                                            bias=qP[:, u:u + 1], scale=1.0)
                                    bi += 1
                            for hp in range(2):
                                # heads hp and hp+2 land in different PE
                                # column groups (strips 2b, 2b+1): interleave
                                # their matmuls so the streams run
                                # concurrently in the array.
                                rts = {}
                                for h in (hp, hp + 2):
                                    rt = relup.tile([128, NS * 512], f16,
                                                    tag="rt", name="rt")
                                    bias_ap = A[:, _C_HQPB + c * 4 + h:
                                                _C_HQPB + c * 4 + h + 1]
                                    nc.vector.tensor_scalar(
                                        rt[:], bsb[:], bias_ap, 0.0,
                                        ALU.add, ALU.max)
                                    rts[h] = rt
                                for dn in range(NS):
                                    n = n0 + dn
                                    for h in (hp, hp + 2):
                                        p_ = (b * H + h) * NBLK + n
                                        g, j = p_ // 32, p_ % 32
                                        first = (c == 0 and n == 0
                                                 and h % 2 == 0)
                                        last = (c == 1 and n == NBLK - 1
                                                and h % 2 == 1)
                                        nc.tensor.matmul(
                                            lp[g][32 * g:32 * g + 32, :],
                                            sb_wz[:, c * 63 + 31 - j:
                                                  c * 63 + 63 - j],
                                            rts[h][:, dn * 512:(dn + 1) * 512],
                                            start=first, stop=last,
                                            tile_position=(0, 32 * g))
                            n0 += NS

                sb_out = outp.tile([128, 512], f32, tag="sbout", name="sbout")
                for g in range(4):
                    nc.vector.tensor_scalar(
                        sb_out[32 * g:32 * g + 32, :],
                        lp[g][32 * g:32 * g + 32, :],
                        A[32 * g:32 * g + 32, _C_ENCB + 6:_C_ENCB + 7],
                        None, ALU.add)
                nc.sync.dma_start(out[:], sb_out[:])
    return nc


def _prep_inputs(uav_feat, task_feat, uw0, ub0, uw1, ub1, uw2, ub2,
                 tw0, tb0, tw1, tb1, tw2, tb2, head_q, fw1, fb1, fw2, fb2):
    f = np.float32
    uav = np.asarray(uav_feat, f)
    task = np.asarray(task_feat, f)
    fw1 = np.asarray(fw1, f)
    fw2 = np.asarray(fw2, f)
    Wq, Wk = fw1[:E], fw1[E:]

    base = np.zeros((128, _C_TOTAL), f)
    base[0:32, _C_UW0:_C_UW0 + 128] = np.asarray(uw0, f)
    base[0:32, _C_TW0:_C_TW0 + 128] = np.asarray(tw0, f)
    base[:, _C_UW1:_C_UW1 + 128] = np.asarray(uw1, f)
    base[:, _C_UW2:_C_UW2 + 128] = np.asarray(uw2, f)
    base[:, _C_TW1:_C_TW1 + 128] = np.asarray(tw1, f)
    base[:, _C_TW2:_C_TW2 + 128] = np.asarray(tw2, f)
    for i, v in enumerate((ub0, ub1, ub2, tb0, tb1, tb2)):
        base[:, _C_ENCB + i] = np.asarray(v, f)
    base[:, _C_ENCB + 6] = np.asarray(fb2, f)[0]
    hq = np.asarray(head_q, f) @ Wq + np.asarray(fb1, f)  # (H, HID)
    for c in range(2):
        for h in range(H):
            base[:, _C_HQPB + c * 4 + h] = hq[h, c * 128:(c + 1) * 128]
    base[:, _C_WQK:_C_WQK + 256] = Wq
    base[:, _C_WQK + 256:_C_WQK + 512] = Wk
    for c in range(2):
        base[:, _C_WZ + c * 63 + 31] = fw2[c * 128:(c + 1) * 128, 0]

    in_maps = []
    for k in range(NCORES):
        b0 = k * BL
        pk = base.copy()
        pk[0:32, _C_UAVT:_C_UAVT + BL * U] = \
            uav[b0:b0 + BL].reshape(BL * U, UAV_DIM).T
        pk[0:32, _C_TASKT:_C_TASKT + BL * T] = \
            task[b0:b0 + BL].reshape(BL * T, TASK_DIM).T
        in_maps.append({"packed": pk})
    return in_maps


def _gather(results):
    outs = []
    for k in range(NCORES):
        r = np.asarray(results[k]["out"], np.float32)  # (128, 512)
        outs.append(r.reshape(BL, H, NBLK, 4, T).reshape(BL, H, U, T))
    return np.concatenate(outs, axis=0)


def kernel(**inputs) -> np.ndarray:
    if "nc" not in _cache:
        _cache["nc"] = _build_nc()
    nc = _cache["nc"]
    in_maps = _prep_inputs(**inputs)
    if os.environ.get("BASS_KERNEL_SIM"):
        # CoreSim can't digest the hand-inserted wait-splitting NoOps; it
        # enforces the multi-wait semantics natively, so run unsplit.
        from concourse.bass_interp import CoreSim
        results = []
        for k in range(NCORES):
            sim = CoreSim(nc)
            for name, arr in in_maps[k].items():
                sim.tensor(name)[:] = arr
            sim.simulate()
            results.append({"out": np.array(sim.tensor("out"))})
    else:
        from concourse.bass_utils import run_bass_kernel_spmd
        if not _cache.get("split"):
            _split_multi_waits(nc)
            _cache["split"] = True
        results = run_bass_kernel_spmd(nc, in_maps, list(range(NCORES))).results
    return _gather(results)
